# revision 49
# baseline (speedup 1.0000x reference)
"""DiT block kernel for Trainium2, 8 NeuronCores, data-parallel over batch.

Each core processes one batch element of x:[8,1024,1024]. All matmuls run in
bf16 with fp32 PSUM accumulation; LayerNorm statistics, softmax denominators,
residual accumulation and the final output stay in fp32.

Layout strategy per core (tokens S=1024, features H=1024, heads 16x64):
  - LayerNorm+modulate in [token, H] layout (tokens on partitions), then
    PE-transpose to xm^T [H, token] for all matmuls.
  - q^T, k^T produced feature-major directly (lhsT = W chunk, rhs = xm^T);
    v produced token-major with a ones-column appended per head so the
    PV matmul also yields the softmax denominator for free.
  - scores^T = k^T.T @ q^T per head; exp with no max-subtraction (scores are
    O(1) by construction: q is pre-scaled by 1/64); softmax denominator from
    the ones row; divide after the PV matmul.
  - MLP: fc1 transposed (h1^T) with fused exact-GELU+bias epilogue, fc2
    accumulated in fp32 SBUF over 8 groups of 512 mlp channels.
"""

import os
import sys

import numpy as np


def _ensure_path():
    for p in ("/opt/trn_rl_repo", "/root/.axon_site/_ro/trn_rl_repo"):
        if os.path.isdir(p) and p not in sys.path:
            sys.path.insert(0, p)


_ensure_path()

import ml_dtypes  # noqa: E402
from contextlib import ExitStack  # noqa: E402

import concourse.bass as bass  # noqa: E402
import concourse.tile as tile  # noqa: E402
from concourse import bacc, mybir  # noqa: E402
from concourse import bass_utils  # noqa: E402
from concourse.masks import make_identity  # noqa: E402

F32 = mybir.dt.float32
BF16 = mybir.dt.bfloat16
AF = mybir.ActivationFunctionType
ALU = mybir.AluOpType

H = 1024
S = 1024
NH = 16
DH = 64
MLP = 4096
B = 8
EPS = 1e-5
P = 128
HC = H // P     # 8 feature chunks
TT = S // P     # 8 token tiles
VW = NH * (DH + 1)  # 1040: v with a ones column appended per head
VH = VW // 2        # 520 (8 heads per half)

_NC = None
LAST_RESULTS = None


def _dram(nc, name, shape, dt, kind="ExternalInput"):
    return nc.dram_tensor(name, list(shape), dt, kind=kind).ap()


def build_nc():
    nc = bacc.Bacc("TRN2", target_bir_lowering=False, debug=False, num_devices=8)

    x_d = _dram(nc, "x", [S, H], F32)
    c_d = _dram(nc, "c", [1, H], F32)
    wc_d = _dram(nc, "wc", [H, 6 * H], BF16)
    bc_d = _dram(nc, "bc", [1, 6 * H], F32)
    wq_d = _dram(nc, "wq", [H, H], BF16)
    bqd_d = _dram(nc, "bqd", [H], F32)      # bq / DH
    wk_d = _dram(nc, "wk", [H, H], BF16)
    bk_d = _dram(nc, "bk", [H], F32)
    wv_d = _dram(nc, "wv", [H, VW], BF16)   # ones col appended per head
    bve_d = _dram(nc, "bve", [1, VW], BF16)  # bv with 1.0 at ones cols
    wo_d = _dram(nc, "wo", [H, H], BF16)
    bor_d = _dram(nc, "bor", [1, H], BF16)
    w1_d = _dram(nc, "w1", [H, MLP], BF16)
    b1_d = _dram(nc, "b1", [MLP], F32)
    w2_d = _dram(nc, "w2", [MLP, H], BF16)
    b2r_d = _dram(nc, "b2r", [1, H], BF16)
    out_d = _dram(nc, "out", [S, H], F32, kind="ExternalOutput")

    # DRAM views with the contraction dim split for partition-major DMA
    wc3 = wc_d.rearrange("(kc p) n -> p kc n", p=P)
    wq3 = wq_d.rearrange("(kc p) n -> p kc n", p=P)
    wk3 = wk_d.rearrange("(kc p) n -> p kc n", p=P)
    wv3 = wv_d.rearrange("(kc p) n -> p kc n", p=P)
    wo3 = wo_d.rearrange("(kc p) n -> p kc n", p=P)
    w13 = w1_d.rearrange("(kc p) n -> p kc n", p=P)
    w23 = w2_d.rearrange("(kc p) n -> p kc n", p=P)

    with ExitStack() as es:
        tc = es.enter_context(tile.TileContext(nc))

        # ---------------- pools (SBUF is the scarce resource) ----------------
        persist = es.enter_context(tc.tile_pool(name="persist", bufs=1))
        # PSUM budget (8 banks): sc = 2x[128,1024] (4 banks) for the big
        # matmul outputs, mm = 4x[128,512] (4) shared by c6/transposes/pv/proj.
        psum = es.enter_context(tc.tile_pool(name="psum", bufs=2, space="PSUM"))
        dramp = es.enter_context(tc.tile_pool(name="dram", bufs=1, space="DRAM"))
        pstat = es.enter_context(tc.tile_pool(name="stat", bufs=4))
        ptmp = es.enter_context(tc.tile_pool(name="tmp", bufs=2))
        pet = es.enter_context(tc.tile_pool(name="etmp", bufs=2))
        wstream = es.enter_context(tc.tile_pool(name="wstream", bufs=3))

        # ---------------- constants ----------------
        ident = persist.tile([P, P], BF16, name="ident")
        make_identity(nc, ident)
        eps_t = persist.tile([P, 1], F32, name="eps_t")
        nc.vector.memset(eps_t, EPS)
        ones_row = persist.tile([1, P], BF16, name="ones_row")
        nc.vector.memset(ones_row, 1.0)
        bqd_t = persist.tile([P, HC], F32, name="bqd_t")
        nc.sync.dma_start(out=bqd_t, in_=bqd_d.rearrange("(oc p) -> p oc", p=P))
        bk_t = persist.tile([P, HC], F32, name="bk_t")
        nc.sync.dma_start(out=bk_t, in_=bk_d.rearrange("(oc p) -> p oc", p=P))
        b1_t = persist.tile([P, MLP // P], F32, name="b1_t")
        nc.sync.dma_start(out=b1_t, in_=b1_d.rearrange("(oc p) -> p oc", p=P))
        bve_sb = persist.tile([1, VW], BF16, name="bve_sb")
        nc.sync.dma_start(out=bve_sb, in_=bve_d)
        bor_sb = persist.tile([1, H], BF16, name="bor_sb")
        nc.sync.dma_start(out=bor_sb, in_=bor_d)
        b2r_sb = persist.tile([1, H], BF16, name="b2r_sb")
        nc.sync.dma_start(out=b2r_sb, in_=b2r_d)

        # ---------------- adaLN: c6 = silu(c) @ Wc + bc ----------------
        ct = persist.tile([P, HC], F32, name="ct")
        nc.sync.dma_start(out=ct, in_=c_d.rearrange("o (j p) -> (o p) j", p=P))
        ct_b = persist.tile([P, HC], BF16, name="ct_b")
        nc.scalar.activation(out=ct_b, in_=ct, func=AF.Silu)
        c6_dram = dramp.tile([1, 6 * H], F32, name="c6_dram")

        def c6_ntile(nt):
            wcs = wstream.tile([P, HC, 512], BF16, tag="wc", bufs=2, name="wcs")
            ps = psum.tile([1, 512], F32, tag="mm", bufs=4, name="c6ps")
            for k in range(HC):
                # per-chunk DMA so the first matmul starts as soon as
                # the first 128 rows of this Wc column block land
                nc.sync.dma_start(
                    out=wcs[:, k, :], in_=wc3[:, k, nt * 512:(nt + 1) * 512]
                )
                nc.tensor.matmul(
                    ps, lhsT=ct_b[:, k:k + 1], rhs=wcs[:, k, :],
                    start=(k == 0), stop=(k == HC - 1),
                )
            bcrow = ptmp.tile([1, 512], F32, tag="bcrow", bufs=1, name="bcrow")
            nc.sync.dma_start(out=bcrow, in_=bc_d[:, nt * 512:(nt + 1) * 512])
            stage = ptmp.tile([1, 512], F32, tag="stage", bufs=1, name="stage")
            nc.vector.tensor_tensor(out=stage, in0=ps, in1=bcrow, op=ALU.add)
            nc.sync.dma_start(
                out=c6_dram[:, nt * 512:(nt + 1) * 512], in_=stage
            )

        for nt in range(4):  # sh_msa, sc_msa
            c6_ntile(nt)

        # column layouts [128, 8] of the modulate vectors: per-partition
        # scalars in the transposed (feature-major) domain
        def cols_from_c6(pool, name, seg, plus1=False):
            t = pool.tile([P, HC], F32, name=name)
            nc.sync.dma_start(
                out=t,
                in_=c6_dram[:, seg * H:(seg + 1) * H].rearrange(
                    "o (j p) -> (o p) j", p=P
                ),
            )
            if plus1:
                nc.scalar.activation(out=t, in_=t, func=AF.Identity, bias=1.0)
            return t

        # residual x, fp32, [128, 8*1024]: token tile i at cols i*1024..
        x_res = persist.tile([P, TT * H], F32, name="x_res")
        for i in range(TT):
            nc.sync.dma_start(
                out=x_res[:, i * H:(i + 1) * H], in_=x_d[i * P:(i + 1) * P, :]
            )

        def ln_stats(src, mv):
            """LN statistics for one tile into mv [128,2] (mean, var)."""
            stats = pstat.tile([P, 2, 6], F32, tag="stats", name="stats")
            for sg in range(2):
                nc.vector.bn_stats(
                    out=stats[:, sg, :], in_=src[:, sg * 512:(sg + 1) * 512]
                )
            nc.vector.bn_aggr(out=mv, in_=stats)

        def ln_normalize(src, out_bf, use_act=True):
            """src [128,1024] f32 SBUF -> out_bf [128,1024] bf16 plain LN."""
            mv = pstat.tile([P, 2], F32, tag="mv", name="mv")
            ln_stats(src, mv)
            sd = pstat.tile([P, 1], F32, tag="sd", name="sd")
            nc.scalar.activation(out=sd, in_=mv[:, 1:2], func=AF.Sqrt, bias=eps_t)
            rstd = pstat.tile([P, 1], F32, tag="rstd", name="rstd")
            nc.vector.reciprocal(rstd, sd)
            if use_act:
                nmr = pstat.tile([P, 1], F32, tag="nmr", name="nmr")
                nc.vector.scalar_tensor_tensor(
                    out=nmr, in0=mv[:, 0:1], scalar=-1.0, in1=rstd,
                    op0=ALU.mult, op1=ALU.mult,
                )
                nc.scalar.activation(
                    out=out_bf, in_=src, func=AF.Identity, bias=nmr, scale=rstd
                )
            else:
                nc.vector.tensor_scalar(
                    out=out_bf, in0=src, scalar1=mv[:, 0:1], scalar2=rstd,
                    op0=ALU.subtract, op1=ALU.mult,
                )

        def transpose_to(xm_b, dstT, i, scT, shT, use_act=True):
            # transpose + modulate: out = in * sc1p^T[hc] + sh^T[hc]
            # (per-partition scalars in the transposed domain); alternate
            # DVE/ACT per chunk so neither engine gates the LN window.
            # use_act=False keeps ACT free of table swaps (e.g. while the
            # attention Exp stream is running).
            for hc in range(HC):
                tp = psum.tile([P, P], BF16, tag="mm", bufs=4, name="tp")
                nc.tensor.transpose(tp, xm_b[:, hc * P:(hc + 1) * P], ident)
                dst = dstT[:, hc * S + i * P: hc * S + (i + 1) * P]
                if hc % 2 == 0 or not use_act:
                    nc.vector.tensor_scalar(
                        out=dst, in0=tp,
                        scalar1=scT[:, hc:hc + 1], scalar2=shT[:, hc:hc + 1],
                        op0=ALU.mult, op1=ALU.add,
                    )
                else:
                    nc.scalar.activation(
                        out=dst, in_=tp, func=AF.Identity,
                        bias=shT[:, hc:hc + 1], scale=scT[:, hc:hc + 1],
                    )

        pxm2 = es.enter_context(tc.tile_pool(name="pxm2", bufs=1))
        xm2T = pxm2.tile([P, HC * S], BF16, name="xm2T")

        with ExitStack() as attn_scope:
            pbm = attn_scope.enter_context(tc.tile_pool(name="bcmsa", bufs=1))
            shT_msa = cols_from_c6(pbm, "shT_msa", 0)
            scT_msa = cols_from_c6(pbm, "scT_msa", 1, plus1=True)
            acts = attn_scope.enter_context(tc.tile_pool(name="acts", bufs=1))
            qT = acts.tile([P, HC * S], BF16, name="qT")
            kT = acts.tile([P, HC * S], BF16, name="kT")
            v_sb = acts.tile([P, TT, VW], BF16, name="v_sb")

            # ---- LN1 + modulate + transpose; then QKV ----
            with tc.tile_pool(name="xmTp", bufs=1) as pxmT:
                xmT = pxmT.tile([P, HC * S], BF16, name="xmT")
                for i in range(TT):
                    xm_b = ptmp.tile([P, H], BF16, tag="xm_b", name="xm_b")
                    ln_normalize(x_res[:, i * H:(i + 1) * H], xm_b)
                    transpose_to(xm_b, xmT, i, scT_msa, shT_msa)

                # q^T / k^T: [hout, tok]
                for dst, w3, bias_t, scale in (
                    (qT, wq3, bqd_t, 1.0 / DH),
                    (kT, wk3, bk_t, 1.0),
                ):
                    for ocH in range(2):
                        wh = wstream.tile([P, HC, 512], BF16, tag="w", name="wh")
                        nc.sync.dma_start(
                            out=wh, in_=w3[:, :, ocH * 512:(ocH + 1) * 512]
                        )
                        for ocl in range(4):
                            oc = ocH * 4 + ocl
                            ps = psum.tile([P, 1024], F32, tag="sc", name="qkps")
                            for nh2 in range(2):
                                for k in range(HC):
                                    nc.tensor.matmul(
                                        ps[:, nh2 * 512:(nh2 + 1) * 512],
                                        lhsT=wh[:, k, ocl * P:(ocl + 1) * P],
                                        rhs=xmT[:, k * S + nh2 * 512:
                                                k * S + (nh2 + 1) * 512],
                                        start=(k == 0), stop=(k == HC - 1),
                                    )
                            nc.scalar.activation(
                                out=dst[:, oc * S: (oc + 1) * S],
                                in_=ps, func=AF.Identity,
                                bias=bias_t[:, oc:oc + 1], scale=scale,
                            )

                # remaining c6 before v so its Wc DMAs start early
                for nt in range(4, 12):
                    c6_ntile(nt)

                # v (token-major, with ones cols): 2 halves of 520 cols
                for vh in range(2):
                    wvh = wstream.tile([P, HC, VH], BF16, tag="w", name="wvh")
                    nc.sync.dma_start(
                        out=wvh, in_=wv3[:, :, vh * VH:(vh + 1) * VH]
                    )
                    for i in range(TT):
                        ps = psum.tile([P, 1024], F32, tag="sc", name="vps")
                        for (n0, n1) in ((0, 512), (512, VH)):
                            pss = ps[:, n0:n1]
                            for k in range(HC):
                                nc.tensor.matmul(
                                    pss,
                                    lhsT=xmT[:, k * S + i * P: k * S + (i + 1) * P],
                                    rhs=wvh[:, k, n0:n1],
                                    start=(k == 0), stop=False,
                                )
                            nc.tensor.matmul(
                                pss, lhsT=ones_row,
                                rhs=bve_sb[:, vh * VH + n0: vh * VH + n1],
                                start=False, stop=True,
                            )
                        nc.vector.tensor_copy(
                            out=v_sb[:, i, vh * VH: (vh + 1) * VH], in_=ps[:, 0:VH]
                        )

            # broadcasts/columns from the second c6 half
            g_msa = pbm.tile([P, H], F32, name="g_msa")
            nc.sync.dma_start(
                out=g_msa, in_=c6_dram[:, 2 * H:3 * H].to_broadcast([P, H])
            )
            shT_mlp = cols_from_c6(persist, "shT_mlp", 3)
            scT_mlp = cols_from_c6(persist, "scT_mlp", 4, plus1=True)
            g_mlp = persist.tile([P, H], F32, name="g_mlp")
            nc.sync.dma_start(
                out=g_mlp, in_=c6_dram[:, 5 * H:6 * H].to_broadcast([P, H])
            )

            # ---- attention ----
            with tc.tile_pool(name="yTp", bufs=1) as pyT:
                yT = pyT.tile([P, HC * S], BF16, name="yT")

                def attn_all(ppt):
                    for h in range(NH):
                        hc = h // 2
                        po = (h % 2) * DH
                        for qh in range(2):
                            q0 = hc * S + qh * 512
                            pT = ppt.tile([P, TT * 512], BF16, tag="pT", name="pT")
                            for jp in range(TT // 2):
                                sp = psum.tile([P, 1024], F32, tag="sc", name="sps")
                                for jj in range(2):
                                    j = jp * 2 + jj
                                    nc.tensor.matmul(
                                        sp[:, jj * 512:(jj + 1) * 512],
                                        lhsT=kT[po:po + DH,
                                                hc * S + j * P: hc * S + (j + 1) * P],
                                        rhs=qT[po:po + DH, q0: q0 + 512],
                                        start=True, stop=True,
                                    )
                                nc.scalar.activation(
                                    out=pT[:, jp * 1024:(jp + 1) * 1024],
                                    in_=sp, func=AF.Exp,
                                )
                            # v block layout per head: [v_64 | ones]: softmax
                            # denominator lands on psum partition 64.
                            pv = psum.tile([DH + 1, 512], F32, tag="mm", bufs=4,
                                           name="pv")
                            for j in range(TT):
                                nc.tensor.matmul(
                                    pv,
                                    lhsT=v_sb[:, j, h * (DH + 1):(h + 1) * (DH + 1)],
                                    rhs=pT[:, j * 512:(j + 1) * 512],
                                    start=(j == 0), stop=(j == TT - 1),
                                )
                            # Copy numerator+denominator out of PSUM right
                            # away so the pv slot frees for the next head;
                            # the reciprocal chain then runs SBUF-only.
                            # (reciprocal_approx_fast corrupts reading PSUM.)
                            den = pet.tile([DH + 1, 512], F32, tag="den", bufs=2,
                                           name="den")
                            nc.vector.tensor_copy(
                                out=den[DH:DH + 1, :], in_=pv[DH:DH + 1, :]
                            )
                            ynum = pet.tile([DH, 512], BF16, tag="ynum", name="ynum")
                            nc.vector.tensor_copy(out=ynum, in_=pv[0:DH, :])
                            # partition_broadcast's ucode reads via gpsimd
                            # core 0 (partitions 0-15): shift the row to
                            # partition 0 with an SBUF->SBUF DMA first.
                            den0 = pet.tile([1, 512], F32, tag="den0", bufs=1,
                                            name="den0")
                            nc.sync.dma_start(out=den0, in_=den[DH:DH + 1, :])
                            nc.vector.reciprocal_approx_fast(out=den0, in_=den0)
                            recipb = pet.tile([DH, 512], F32, tag="recipb",
                                              bufs=1, name="recipb")
                            nc.gpsimd.partition_broadcast(recipb, den0)
                            nc.vector.tensor_tensor(
                                out=ynum, in0=ynum, in1=recipb, op=ALU.mult
                            )
                            nc.sync.dma_start(
                                out=yT[po:po + DH, q0:q0 + 512], in_=ynum
                            )

                with tc.tile_pool(name="pthead", bufs=2) as ppt:
                    attn_all(ppt)

                # ---- out-proj (i-outer, both Wo halves resident) with
                # LN2 of tile i interleaved right behind proj of tile i ----
                woh = []
                for nh2 in range(2):
                    w = wstream.tile([P, HC, 512], BF16, tag="w", name="woh")
                    nc.sync.dma_start(
                        out=w, in_=wo3[:, :, nh2 * 512:(nh2 + 1) * 512]
                    )
                    woh.append(w)
                for i in range(TT):
                    ps = psum.tile([P, 1024], F32, tag="sc", name="prps")
                    for nh2 in range(2):
                        pss = ps[:, nh2 * 512:(nh2 + 1) * 512]
                        for k in range(HC):
                            nc.tensor.matmul(
                                pss,
                                lhsT=yT[:, k * S + i * P: k * S + (i + 1) * P],
                                rhs=woh[nh2][:, k, :],
                                start=(k == 0), stop=False,
                            )
                        nc.tensor.matmul(
                            pss, lhsT=ones_row,
                            rhs=bor_sb[:, nh2 * 512:(nh2 + 1) * 512],
                            start=False, stop=True,
                        )
                    rt = ptmp.tile([P, H], F32, tag="rt", bufs=1, name="rt")
                    nc.vector.tensor_tensor(
                        out=rt, in0=ps, in1=g_msa, op=ALU.mult,
                    )
                    xsl = x_res[:, i * H:(i + 1) * H]
                    nc.gpsimd.tensor_tensor(out=xsl, in0=xsl, in1=rt, op=ALU.add)
                    xm_b = ptmp.tile([P, H], BF16, tag="xm_b", name="xm2_b")
                    ln_normalize(xsl, xm_b)
                    transpose_to(xm_b, xm2T, i, scT_mlp, shT_mlp)

        # ---- MLP ----
        # fc1 streams W1 in 512-col halves into 1024-wide h1 groups; fc2
        # accumulates 8 chunks per group in PSUM, fp32 adds into acc only
        # at group granularity (4 groups).
        with tc.tile_pool(name="mlp", bufs=1) as pmlp, \
                tc.tile_pool(name="h1p", bufs=2) as ph1, \
                tc.tile_pool(name="outp", bufs=2) as pout:
            NG = 4
            GK = 8               # 8 k-chunks of 128 per group (1024 wide)
            acc = pmlp.tile([P, TT * H], F32, name="acc")
            for g in range(NG):
                h1gT = ph1.tile([P, GK, S], BF16, tag="h1", name="h1gT")
                for wh2 in range(2):
                    w1g = wstream.tile([P, HC, 512], BF16, tag="w", name="w1g")
                    c0 = g * 1024 + wh2 * 512
                    nc.sync.dma_start(out=w1g, in_=w13[:, :, c0:c0 + 512])
                    for mcl4 in range(4):
                        mcl = wh2 * 4 + mcl4
                        mc = g * GK + mcl
                        ps = psum.tile([P, 1024], F32, tag="sc", name="f1ps")
                        for nh2 in range(2):
                            for k in range(HC):
                                nc.tensor.matmul(
                                    ps[:, nh2 * 512:(nh2 + 1) * 512],
                                    lhsT=w1g[:, k, mcl4 * P:(mcl4 + 1) * P],
                                    rhs=xm2T[:, k * S + nh2 * 512:
                                             k * S + (nh2 + 1) * 512],
                                    start=(k == 0), stop=(k == HC - 1),
                                )
                        nc.scalar.activation(
                            out=h1gT[:, mcl, :],
                            in_=ps, func=AF.Gelu,
                            bias=b1_t[:, mc: mc + 1],
                        )
                w2ga = []
                for wh2 in range(2):
                    w2g = wstream.tile([P, 4, H], BF16, tag="w", name="w2g")
                    nc.sync.dma_start(
                        out=w2g,
                        in_=w23[:, g * GK + wh2 * 4: g * GK + (wh2 + 1) * 4, :],
                    )
                    w2ga.append(w2g)
                for i in range(TT):
                    ps = psum.tile([P, 1024], F32, tag="sc", name="f2ps")
                    for nh2 in range(2):
                        pss = ps[:, nh2 * 512:(nh2 + 1) * 512]
                        for mcl in range(GK):
                            nc.tensor.matmul(
                                pss, lhsT=h1gT[:, mcl, i * P:(i + 1) * P],
                                rhs=w2ga[mcl // 4][:, mcl % 4,
                                                   nh2 * 512:(nh2 + 1) * 512],
                                start=(mcl == 0),
                                stop=(g != NG - 1 and mcl == GK - 1),
                            )
                        if g == NG - 1:
                            nc.tensor.matmul(
                                pss, lhsT=ones_row,
                                rhs=b2r_sb[:, nh2 * 512:(nh2 + 1) * 512],
                                start=False, stop=True,
                            )
                    asl = acc[:, i * H:(i + 1) * H]
                    if g == 0:
                        nc.vector.tensor_copy(out=asl, in_=ps)
                    elif g < NG - 1:
                        nc.vector.tensor_tensor(out=asl, in0=asl, in1=ps, op=ALU.add)
                    else:
                        ot = pout.tile([P, H], F32, tag="ot", name="ot")
                        nc.vector.tensor_tensor(out=ot, in0=asl, in1=ps, op=ALU.add)
                        nc.vector.tensor_tensor(
                            out=ot, in0=ot, in1=g_mlp, op=ALU.mult
                        )
                        nc.gpsimd.tensor_tensor(
                            out=ot, in0=ot, in1=x_res[:, i * H:(i + 1) * H],
                            op=ALU.add,
                        )
                        nc.sync.dma_start(
                            out=out_d[i * P:(i + 1) * P, :], in_=ot
                        )

    nc.compile()
    return nc


def get_nc():
    global _NC
    if _NC is None:
        _NC = build_nc()
    return _NC


def make_in_maps(inputs):
    bf = ml_dtypes.bfloat16
    x = np.ascontiguousarray(inputs["x"], dtype=np.float32)
    c = np.ascontiguousarray(inputs["c"], dtype=np.float32)
    Wv = np.asarray(inputs["Wv"], dtype=np.float32)
    bv = np.asarray(inputs["bv"], dtype=np.float32)
    wv_ext = np.zeros((H, VW), dtype=np.float32)
    bv_ext = np.zeros((1, VW), dtype=np.float32)
    for h in range(NH):
        wv_ext[:, h * (DH + 1):h * (DH + 1) + DH] = Wv[:, h * DH:(h + 1) * DH]
        bv_ext[0, h * (DH + 1):h * (DH + 1) + DH] = bv[h * DH:(h + 1) * DH]
        bv_ext[0, h * (DH + 1) + DH] = 1.0

    shared = {
        "wc": np.asarray(inputs["Wc"], dtype=np.float32).astype(bf),
        "bc": np.asarray(inputs["bc"], dtype=np.float32).reshape(1, 6 * H),
        "wq": np.asarray(inputs["Wq"], dtype=np.float32).astype(bf),
        "bqd": (np.asarray(inputs["bq"], dtype=np.float32) / DH),
        "wk": np.asarray(inputs["Wk"], dtype=np.float32).astype(bf),
        "bk": np.asarray(inputs["bk"], dtype=np.float32),
        "wv": wv_ext.astype(bf),
        "bve": bv_ext.astype(bf),
        "wo": np.asarray(inputs["Wo"], dtype=np.float32).astype(bf),
        "bor": np.asarray(inputs["bo"], dtype=np.float32).reshape(1, H).astype(bf),
        "w1": np.asarray(inputs["W1"], dtype=np.float32).astype(bf),
        "b1": np.asarray(inputs["b1"], dtype=np.float32),
        "w2": np.asarray(inputs["W2"], dtype=np.float32).astype(bf),
        "b2r": np.asarray(inputs["b2"], dtype=np.float32).reshape(1, H).astype(bf),
    }
    in_maps = []
    for b in range(B):
        m = dict(shared)
        m["x"] = x[b]
        m["c"] = c[b:b + 1]
        in_maps.append(m)
    return in_maps


def kernel(**inputs) -> np.ndarray:
    global LAST_RESULTS
    nc = get_nc()
    in_maps = make_in_maps(inputs)
    res = bass_utils.run_bass_kernel_spmd(nc, in_maps, core_ids=list(range(B)))
    LAST_RESULTS = res
    out = np.stack([res.results[b]["out"] for b in range(B)], axis=0)
    return out.astype(np.float32)


if __name__ == "__main__":
    build_nc()
    print("built and compiled OK")
